# revision 23
# baseline (speedup 1.0000x reference)
"""Bass/Trainium2 kernel for nn_BerpXposMultiHeadedAttention (8-core SPMD).

Sharding: data-parallel over batch (4 batches x 2 cores) x tensor-parallel over
heads (4 heads per core).  Each core computes its 4 heads of flash-style xpos
attention for its batch plus the row-sharded partial out-projection; the host
sums the two partials per batch (the "all-reduce") and adds the output bias.

v2 design notes (from NTFF trace analysis of the v1 baseline):
- v1 spent 74% of its time with the PE clock HAM-throttled to 1.2 GHz and
  another ~50us/rep on full-width DVE reciprocals.  v2 restructures for a
  dense PE stream: weights/tables load once in a preamble, x loads are 12
  full-width DMAs at rep start, and next-strip projections are emitted
  between flash head-pairs so the PE always has ready matmuls.
- Scores for a head PAIR go into one [128,1024] PSUM tile (head even in cols
  0:512 from PE rows 0:63, head odd in cols 512:1024 from rows 64:127) so
  consecutive fp32r matmuls alternate row groups and overlap in the array.
- exp (ACT) and P@V (PE) interleave per s-block: P@V for block s starts as
  soon as its exp lands, so pt tiles need only ~6 bufs and ACT/PE ping-pong
  disappears.
- Softmax normalization: denominator row -> reciprocal_approx_fast (5x
  cheaper than reciprocal, ~18 bits) -> DRAM round-trip broadcast -> one
  mul per head on GpSimd.  Drains are deferred two head-pairs to hide the
  DMA latency.
- cos|sin xpos tables are packed per strip so the post-projection combine is
  one [128,1024] DVE mul + one [128,512] add instead of three ops.
- Output is written f16 (halves the out DMA); host accumulates in f32.
"""

import sys

sys.path.insert(0, "/opt/trn_rl_repo")

import contextlib

import numpy as np

import concourse.bacc as bacc
import concourse.bass as bass
import concourse.tile as tile
from concourse import mybir
from concourse.bass_utils import run_bass_kernel_spmd

# Problem constants (hardcoded per the task contract).
B = 4
L = 2048
EMBED = 512
HEADS = 8
HD = 64
SCALE_BASE = 512
NEG = -1e9
N_CORES = 8
HPC = 4           # heads per core
TB = 512          # t-block (strip) width
NT = L // 128     # 16 t-chunks
NS = L // 128     # 16 s-chunks
NSTRIP = L // TB  # 4 strips
VW = 328          # v_aug tile width (4 heads x 65 + 68 pad)

F32 = mybir.dt.float32
F32R = mybir.dt.float32r
F16 = mybir.dt.float16
BF16 = mybir.dt.bfloat16

# Deinterleave permutation of a 64-wide head dim: evens then odds.
_PERM64 = np.concatenate([np.arange(0, HD, 2), np.arange(1, HD, 2)])


def _xpos_tables():
    """Host-side xpos cos/sin tables in the permuted [d, t] layout.

    Returns (cq, sq, ck, sk), each [128, L] float32 (two heads' worth of rows,
    identical per head).  The 1/sqrt(HD) score scale is folded into the q pair.
    """
    d = HD
    base = ((np.arange(0, d, 2, dtype=np.float32) + np.float32(0.4 * d))
            / np.float32(1.4 * d)).astype(np.float32)                    # [32]
    min_pos = -(L // 2)
    power = (np.arange(min_pos, L + min_pos, dtype=np.float32)
             / np.float32(SCALE_BASE))                                   # [L]
    scale = (base[None, :] ** power[:, None]).astype(np.float32)         # [L, 32]
    half = d // 2
    inv_freq = (1.0 / (10000.0 ** (np.arange(half, dtype=np.float32) / half))
                ).astype(np.float32)
    sinusoid = np.arange(L, dtype=np.float32)[:, None] * inv_freq[None, :]
    sin = np.sin(sinusoid).astype(np.float32)
    cos = np.cos(sinusoid).astype(np.float32)

    def pack(cs, ss, fold):
        cs = (cs * fold).astype(np.float32)
        ss = (ss * fold).astype(np.float32)
        # permuted layout: rows 0:32 <- even orig dims, rows 32:64 <- odd.
        cos_p = np.concatenate([cs.T, cs.T], axis=0)      # [64, L]
        sin_p = np.concatenate([-ss.T, ss.T], axis=0)     # [64, L]
        return (np.concatenate([cos_p, cos_p], axis=0).astype(np.float32),
                np.concatenate([sin_p, sin_p], axis=0).astype(np.float32))

    inv_scale = (1.0 / scale).astype(np.float32)
    cq, sq = pack(cos * scale, sin * scale, np.float32(HD ** -0.5))
    ck, sk = pack(cos * inv_scale, sin * inv_scale, np.float32(1.0))
    return cq, sq, ck, sk


def _build_program(causal: bool, use_mask: bool, has_bias: bool, reps: int = 1):
    nc = bacc.Bacc("TRN2", target_bir_lowering=False, debug=False,
                   num_devices=N_CORES)

    # ---- DRAM I/O -------------------------------------------------------
    xqT = nc.dram_tensor("xqT", [513, L], F16, kind="ExternalInput")
    xkT = nc.dram_tensor("xkT", [513, L], F16, kind="ExternalInput")
    xvT = nc.dram_tensor("xvT", [513, L], F16, kind="ExternalInput")
    wqcT = nc.dram_tensor("wqcT", [513, 256], BF16, kind="ExternalInput")
    wqsT = nc.dram_tensor("wqsT", [513, 256], BF16, kind="ExternalInput")
    wkcT = nc.dram_tensor("wkcT", [513, 256], BF16, kind="ExternalInput")
    wksT = nc.dram_tensor("wksT", [513, 256], BF16, kind="ExternalInput")
    wvT = nc.dram_tensor("wvT", [513, 256], BF16, kind="ExternalInput")
    woT = nc.dram_tensor("woT", [256, EMBED], BF16, kind="ExternalInput")
    # packed per-strip [cos | sin] tables: cols [tb*1024, tb*1024+512) = cos
    # strip tb, [+512, +1024) = sin strip tb.
    tqD = nc.dram_tensor("tq", [128, 2 * L], F32, kind="ExternalInput")
    tkD = nc.dram_tensor("tk", [128, 2 * L], F32, kind="ExternalInput")
    triD = nc.dram_tensor("tri", [128, 128], F32, kind="ExternalInput")
    tri01D = nc.dram_tensor("tri01", [128, 128], BF16, kind="ExternalInput")
    maskD = None
    if use_mask:
        maskD = nc.dram_tensor("maskT", [L, L], F32, kind="ExternalInput")
    outp = nc.dram_tensor("outp", [L, EMBED], F16, kind="ExternalOutput")

    xin = {"q": xqT, "k": xkT, "v": xvT}
    win = {"qc": wqcT, "qs": wqsT, "kc": wkcT, "ks": wksT, "v": wvT}

    with tile.TileContext(nc) as tc:
        with contextlib.ExitStack() as ctx:
            consts = ctx.enter_context(tc.tile_pool(name="consts", bufs=1))
            xpool = ctx.enter_context(tc.tile_pool(name="xpool", bufs=1))
            qkpool = ctx.enter_context(tc.tile_pool(name="qkpool", bufs=1))
            vpool = ctx.enter_context(tc.tile_pool(name="vpool", bufs=NS))
            tmp = ctx.enter_context(tc.tile_pool(name="tmp", bufs=3))
            ptpool = ctx.enter_context(tc.tile_pool(name="ptpool", bufs=8))
            npool = ctx.enter_context(tc.tile_pool(name="npool", bufs=6))
            opool = ctx.enter_context(tc.tile_pool(name="opool", bufs=2))
            drpool = ctx.enter_context(
                tc.tile_pool(name="drpool", bufs=6, space="DRAM"))
            mpool = None
            if use_mask:
                mpool = ctx.enter_context(tc.tile_pool(name="mpool", bufs=NS + 2))
            ps_big = ctx.enter_context(tc.tile_pool(name="ps_big", bufs=3,
                                                    space="PSUM"))
            ps_sm = ctx.enter_context(tc.tile_pool(name="ps_sm", bufs=2,
                                                   space="PSUM"))

            # ---- preamble: everything that doesn't change per rep -------
            ones_sb = None
            if has_bias:
                ones_sb = consts.tile([1, L], F16, tag="ones")
                nc.sync.dma_start(ones_sb[:], xqT[512:513, :])
            tq_sb = consts.tile([128, 2 * L], F32, tag="tq")
            tk_sb = consts.tile([128, 2 * L], F32, tag="tk")

            w_sb = {}

            def _load_w(nm):
                chunks = []
                for c in range(4):
                    t = consts.tile([128, 256], BF16, tag=f"w{nm}{c}",
                                    name=f"w{nm}{c}")
                    nc.sync.dma_start(t[:], win[nm][c * 128:(c + 1) * 128, :])
                    chunks.append(t)
                bt = None
                if has_bias:
                    bt = consts.tile([1, 256], BF16, tag=f"w{nm}b",
                                     name=f"w{nm}b")
                    nc.sync.dma_start(bt[:], win[nm][512:513, :])
                w_sb[nm] = (chunks, bt)

            # consumption order: strip-0 projections need q/k/v weights and
            # the strip-0 table slices before anything else.
            _load_w("qc")
            _load_w("qs")
            nc.sync.dma_start(tq_sb[:, 0:1024], tqD[:, 0:1024])
            _load_w("kc")
            _load_w("ks")
            nc.sync.dma_start(tk_sb[:, 0:1024], tkD[:, 0:1024])
            _load_w("v")
            tri01_sb = consts.tile([128, 128], BF16, tag="tri01")
            if causal:
                nc.sync.dma_start(tri01_sb[:], tri01D[:])
            for tb in range(1, NSTRIP):
                cs = slice(tb * 1024, (tb + 1) * 1024)
                nc.sync.dma_start(tq_sb[:, cs], tqD[:, cs])
                nc.sync.dma_start(tk_sb[:, cs], tkD[:, cs])
            wo_sb = []
            for c in range(2):
                t = consts.tile([128, EMBED], BF16, tag=f"wo{c}", name=f"wo{c}")
                nc.sync.dma_start(t[:], woT[c * 128:(c + 1) * 128, :])
                wo_sb.append(t)

            attnT = [consts.tile([128, L], BF16, tag=f"attnT{c}",
                                 name=f"attnT{c}") for c in range(2)]

            def body():
                x_sb = {}
                for nm in ("q", "k", "v"):
                    chunks = []
                    for c in range(4):
                        t = xpool.tile([128, L], F16, tag=f"x{nm}{c}",
                                       name=f"x{nm}{c}")
                        nc.sync.dma_start(t[:, 0:TB],
                                          xin[nm][c * 128:(c + 1) * 128, 0:TB])
                        chunks.append(t)
                    x_sb[nm] = chunks
                for nm in ("q", "k", "v"):
                    for c in range(4):
                        nc.sync.dma_start(
                            x_sb[nm][c][:, TB:L],
                            xin[nm][c * 128:(c + 1) * 128, TB:L])

                qTt = [[None] * NSTRIP for _ in range(2)]  # [pair][tb]
                kTt = [[None] * NSTRIP for _ in range(2)]
                vaug = [None] * NS
                pending = []

                def proj_qk(nm, tb):
                    wc, wcb = w_sb[nm + "c"]
                    ws, wsb = w_sb[nm + "s"]
                    tab = tq_sb if nm == "q" else tk_sb
                    ts = slice(tb * TB, (tb + 1) * TB)
                    dst = qTt if nm == "q" else kTt
                    for e in range(2):
                        es = slice(e * 128, (e + 1) * 128)
                        ps = ps_big.tile([128, 1024], F32, tag="s",
                                         name=f"ps_{nm}{e}_{tb}")
                        for c in range(4):
                            nc.tensor.matmul(ps[:, 0:TB], wc[c][:, es],
                                             x_sb[nm][c][:, ts],
                                             start=(c == 0),
                                             stop=(c == 3 and not has_bias))
                        if has_bias:
                            nc.tensor.matmul(ps[:, 0:TB], wcb[:, es],
                                             ones_sb[:, ts],
                                             start=False, stop=True)
                        for c in range(4):
                            nc.tensor.matmul(ps[:, TB:1024], ws[c][:, es],
                                             x_sb[nm][c][:, ts],
                                             start=(c == 0),
                                             stop=(c == 3 and not has_bias))
                        if has_bias:
                            nc.tensor.matmul(ps[:, TB:1024], wsb[:, es],
                                             ones_sb[:, ts],
                                             start=False, stop=True)
                        t12 = tmp.tile([128, 1024], F32, tag="t12",
                                       name=f"t12{nm}{e}{tb}")
                        nc.vector.tensor_mul(
                            t12[:], ps[:],
                            tab[:, tb * 1024:(tb + 1) * 1024])
                        ot = qkpool.tile([128, TB], F16, tag=f"{nm}T{e}_{tb}",
                                         name=f"{nm}T{e}_{tb}")
                        nc.vector.tensor_add(ot[:], t12[:, 0:TB],
                                             t12[:, TB:1024])
                        dst[e][tb] = ot

                def proj_v(tb):
                    wv, wvb = w_sb["v"]
                    for j in range(4):
                        si = tb * 4 + j
                        js = slice(si * 128, (si + 1) * 128)
                        ps = ps_sm.tile([128, 256], F32, tag="pv",
                                        name=f"ps_v{si}")
                        for c in range(4):
                            nc.tensor.matmul(ps[:], x_sb["v"][c][:, js], wv[c][:],
                                             start=(c == 0),
                                             stop=(c == 3 and not has_bias))
                        if has_bias:
                            nc.tensor.matmul(
                                ps[:], ones_sb[:, js],
                                wvb[:], start=False, stop=True)
                        va = vpool.tile([128, VW], BF16, tag="vaug",
                                        name=f"vaug{si}")
                        va3 = va[:, 0:HPC * 65].rearrange("p (h c) -> p h c", c=65)
                        nc.vector.tensor_copy(
                            va3[:, :, 0:64],
                            ps[:].rearrange("p (h d) -> p h d", d=64))
                        nc.gpsimd.memset(va3[:, :, 64:65], 1.0)
                        nc.gpsimd.memset(va[:, HPC * 65:VW], 0.0)
                        vaug[si] = va

                def launch_norm(po, ht, hr, T):
                    poc = npool.tile([65, TB], F32, tag="poc",
                                     name=f"poc{T}h{ht}{hr}", bufs=6)
                    nc.vector.tensor_copy(poc[:], po[0:65, :])
                    dbn = drpool.tile([1, TB], F32, tag="dbn",
                                      name=f"dbn{T}h{ht}{hr}")
                    nc.sync.dma_start(dbn[:], poc[64:65, :])
                    sums = npool.tile([64, TB], F32, tag="sums",
                                      name=f"sums{T}h{ht}{hr}", bufs=6)
                    bcast = bass.AP(tensor=dbn[:].tensor, offset=dbn[:].offset,
                                    ap=[[0, 64], [1, TB]])
                    nc.sync.dma_start(sums[:], bcast)
                    pending.append((poc, sums, ht, hr, T))

                def drain_norms(keep=0):
                    while len(pending) > keep:
                        poc, sums, ht, hr, T = pending.pop(0)
                        tcols = slice(T * TB, (T + 1) * TB)
                        rec = npool.tile([64, TB], F32, tag="recr",
                                         name=f"rec{T}x{ht}{hr}", bufs=2)
                        nc.vector.reciprocal_approx_fast(rec[:], sums[:])
                        if hr == 0:
                            nc.gpsimd.tensor_mul(attnT[ht][0:64, tcols],
                                                 poc[0:64, :], rec[:])
                        else:
                            stag = npool.tile([64, TB], BF16, tag="stag",
                                              name=f"stag{T}x{ht}", bufs=2)
                            nc.gpsimd.tensor_mul(stag[:], poc[0:64, :], rec[:])
                            nc.sync.dma_start(attnT[ht][64:128, tcols], stag[:])

                def flash_strip(T):
                    nsig = 4 * T + 4 if causal else NS
                    mtiles = None
                    if use_mask:
                        mtiles = []
                        for si in range(nsig):
                            mt = mpool.tile([128, TB], F32, tag="mask",
                                            name=f"m{T}_{si}")
                            nc.sync.dma_start(
                                mt[:], maskD[si * 128:(si + 1) * 128,
                                             T * TB:(T + 1) * TB])
                            mtiles.append(mt)
                    for hp in range(2):
                        drain_norms(keep=4 if T < NSTRIP - 1 else 2)
                        # keep the PE fed with next-strip projection work
                        # while ACT chews on this pair's exps.
                        if T < NSTRIP - 1:
                            if hp == 0:
                                proj_qk("q", T + 1)
                            else:
                                proj_v(T + 1)
                                proj_qk("k", T + 1)
                        elif hp == 1:
                            # strip 2's second pair must be normalized before
                            # its out batch reads those attnT columns.
                            drain_norms(keep=1)
                            out_proj_batch(2)
                            out_proj_batch(1)
                        hA, hB = 2 * hp, 2 * hp + 1
                        poA = ps_sm.tile([128, TB], F32, tag="pv",
                                         name=f"poA{T}p{hp}")
                        poB = ps_sm.tile([128, TB], F32, tag="pv",
                                         name=f"poB{T}p{hp}")
                        for sig in range(nsig):
                            ps2 = ps_big.tile([128, 1024], F32, tag="s",
                                              name=f"S{T}p{hp}s{sig}")
                            pt = ptpool.tile([128, 1024], BF16, tag="pt",
                                             name=f"P{T}p{hp}s{sig}")
                            j = sig - 4 * T
                            coff = 0
                            if causal and j >= 0:
                                ncols = max(TB - j * 128, 256)
                                coff = TB - ncols
                            kT_s = kTt[hp][sig // 4]
                            qT_s = qTt[hp][T]
                            ss = slice((sig % 4) * 128, (sig % 4 + 1) * 128)
                            nc.tensor.matmul(ps2[:, coff:TB],
                                             kT_s[0:64, ss],
                                             qT_s[0:64, coff:TB],
                                             start=True, stop=True)
                            nc.tensor.matmul(ps2[:, TB + coff:1024],
                                             kT_s[64:128, ss],
                                             qT_s[64:128, coff:TB],
                                             start=True, stop=True)
                            if use_mask:
                                for u in range(2):
                                    sl = slice(u * TB, (u + 1) * TB)
                                    nc.vector.tensor_add(ps2[:, sl], ps2[:, sl],
                                                         mtiles[sig][:])
                            # exp only the matmul-written region: reading the
                            # unwritten [0:coff) would race the pool slot's
                            # next occupant (same-bank PE-W vs ACT-R).
                            pt3 = pt[:].rearrange("p (u t) -> p u t", u=2)
                            ps3 = ps2[:].rearrange("p (u t) -> p u t", u=2)
                            nc.scalar.activation(pt3[:, :, coff:TB],
                                                 ps3[:, :, coff:TB],
                                                 mybir.ActivationFunctionType.Exp)
                            for u in range(2):
                                if causal and j >= 0:
                                    sl = slice(u * TB + j * 128,
                                               u * TB + (j + 1) * 128)
                                    nc.gpsimd.tensor_mul(pt[:, sl], pt[:, sl],
                                                         tri01_sb[:])
                                if causal and 1 <= j <= 3:
                                    nc.gpsimd.memset(
                                        pt[:, u * TB:u * TB + j * 128], 0.0)
                            nc.tensor.matmul(
                                poA[:], vaug[sig][:, hA * 65:hA * 65 + 128],
                                pt[:, 0:TB],
                                start=(sig == 0), stop=(sig == nsig - 1))
                            nc.tensor.matmul(
                                poB[:], vaug[sig][:, hB * 65:hB * 65 + 128],
                                pt[:, TB:1024],
                                start=(sig == 0), stop=(sig == nsig - 1))
                        launch_norm(poA, hp, 0, T)
                        launch_norm(poB, hp, 64, T)
                    # attnT cols for strip T-2 are fully normalized once this
                    # strip's pairs have drained; emit its out batch now.
                    if T == 2:
                        out_proj_batch(0)

                def out_proj_batch(ob):
                    osb = opool.tile([128, 4 * EMBED], F16, tag="osb",
                                     name=f"osb{ob}")
                    for i in range(4):
                        tau = ob * 4 + i
                        ps = ps_sm.tile([128, EMBED], F32, tag="pv",
                                        name=f"ps_o{tau}")
                        for c in range(2):
                            nc.tensor.matmul(
                                ps[:], attnT[c][:, tau * 128:(tau + 1) * 128],
                                wo_sb[c][:], start=(c == 0), stop=(c == 1))
                        nc.vector.tensor_copy(
                            osb[:, i * EMBED:(i + 1) * EMBED], ps[:])
                    dst = outp[ob * 512:(ob + 1) * 512, :].rearrange(
                        "(i p) e -> p i e", i=4)
                    nc.sync.dma_start(
                        dst, osb[:].rearrange("p (i e) -> p i e", i=4))

                proj_qk("q", 0)
                proj_qk("k", 0)
                proj_v(0)
                for T in range(NSTRIP):
                    flash_strip(T)
                drain_norms()
                out_proj_batch(3)

            if reps > 1:
                inner = 4 if reps % 4 == 0 else (2 if reps % 2 == 0 else 1)
                with tc.For_i(0, reps // inner, 1,
                              hint_engines=(mybir.EngineType.PE,
                                            mybir.EngineType.Activation,
                                            mybir.EngineType.DVE,
                                            mybir.EngineType.SP,
                                            mybir.EngineType.Pool)):
                    for _ in range(inner):
                        body()
            else:
                body()

    nc.compile()
    return nc


_PROGRAM_CACHE = {}


def get_program(causal: bool, use_mask: bool, has_bias: bool, reps: int = 1):
    key = (causal, use_mask, has_bias, reps)
    if key not in _PROGRAM_CACHE:
        _PROGRAM_CACHE[key] = _build_program(causal, use_mask, has_bias, reps)
    return _PROGRAM_CACHE[key]


def _prep_in_maps(query, key, value, key_padding_mask, attn_mask,
                  Wq, bq, Wk, bk, Wv, bv, Wo, bo, use_mask, has_bias):
    """Build the 8 per-core input dicts."""
    import ml_dtypes
    cq, sq, ck, sk = _xpos_tables()
    # pack per strip: [cos strip | sin strip] -> [128, 2L]
    tq = np.empty((128, 2 * L), np.float32)
    tk = np.empty((128, 2 * L), np.float32)
    for tb in range(NSTRIP):
        ts = slice(tb * TB, (tb + 1) * TB)
        tq[:, tb * 1024:tb * 1024 + TB] = cq[:, ts]
        tq[:, tb * 1024 + TB:(tb + 1) * 1024] = sq[:, ts]
        tk[:, tb * 1024:tb * 1024 + TB] = ck[:, ts]
        tk[:, tb * 1024 + TB:(tb + 1) * 1024] = sk[:, ts]
    tri = np.where(np.arange(128)[None, :] >= np.arange(128)[:, None],
                   np.float32(0.0), np.float32(NEG)).astype(np.float32)
    import ml_dtypes as _mld
    tri01 = np.where(np.arange(128)[None, :] >= np.arange(128)[:, None],
                     np.float32(1.0), np.float32(0.0)).astype(_mld.bfloat16)

    def aug_x(x):
        a = np.empty((513, L), np.float16)
        a[0:512] = np.asarray(x, np.float32).T.astype(np.float16)
        a[512] = np.float16(1.0)
        return a

    xqTs = [aug_x(query[b]) for b in range(B)]
    xkTs = [aug_x(key[b]) for b in range(B)]
    xvTs = [aug_x(value[b]) for b in range(B)]

    masks = None
    if use_mask:
        am = np.asarray(attn_mask, np.float32)
        kp = np.asarray(key_padding_mask)
        masks = []
        for b in range(B):
            m = am.copy()
            if kp[b].any():
                m = m + np.where(kp[b], np.float32(-1e30),
                                 np.float32(0.0))[None, :]
            masks.append(np.ascontiguousarray(m.T.astype(np.float32)))

    Wq = np.asarray(Wq, np.float32); bq = np.asarray(bq, np.float32)
    Wk = np.asarray(Wk, np.float32); bk = np.asarray(bk, np.float32)
    Wv = np.asarray(Wv, np.float32); bv = np.asarray(bv, np.float32)
    Wo = np.asarray(Wo, np.float32)

    in_maps = []
    for core in range(N_CORES):
        b, hg = core // 2, core % 2
        hs = hg * HPC
        idx_p = np.concatenate(
            [hs * HD + hl * HD + _PERM64 for hl in range(HPC)])
        # sin-projection rows: within each head's 64-block, row r <- r XOR 32
        xor = (np.arange(256).reshape(HPC, HD)[:, (np.arange(HD) ^ 32)]
               ).reshape(-1)
        idx_s = idx_p[xor]
        idx_v = hs * HD + np.arange(HPC * HD)

        def aug_w(W, bias, idx):
            a = np.empty((513, 256), np.float32)
            a[0:512] = np.ascontiguousarray(W[idx, :].T)
            a[512] = bias[idx]
            return a.astype(ml_dtypes.bfloat16)

        m = {
            "xqT": xqTs[b], "xkT": xkTs[b], "xvT": xvTs[b],
            "wqcT": aug_w(Wq, bq, idx_p),
            "wqsT": aug_w(Wq, bq, idx_s),
            "wkcT": aug_w(Wk, bk, idx_p),
            "wksT": aug_w(Wk, bk, idx_s),
            "wvT": aug_w(Wv, bv, idx_v),
            "woT": np.ascontiguousarray(Wo[:, idx_v].T).astype(ml_dtypes.bfloat16),
            "tq": tq, "tk": tk,
            "tri": tri, "tri01": tri01,
        }
        if use_mask:
            m["maskT"] = masks[b]
        in_maps.append(m)
    return in_maps


def classify_mask(attn_mask, key_padding_mask):
    am = np.asarray(attn_mask, np.float32)
    kp = np.asarray(key_padding_mask)
    if not kp.any():
        causal = np.where(
            np.tril(np.ones((L, L), bool)), np.float32(0.0),
            np.float32(NEG)).astype(np.float32)
        if np.array_equal(am, causal):
            return True, False
        if not am.any():
            return False, False
    return False, True


def kernel(query, key, value, key_padding_mask, attn_mask,
           Wq, bq, Wk, bk, Wv, bv, Wo, bo):
    causal, use_mask = classify_mask(attn_mask, key_padding_mask)
    has_bias = bool(np.asarray(bq).any() or np.asarray(bk).any()
                    or np.asarray(bv).any())
    nc = get_program(causal, use_mask, has_bias, reps=1)
    in_maps = _prep_in_maps(query, key, value, key_padding_mask, attn_mask,
                            Wq, bq, Wk, bk, Wv, bv, Wo, bo, use_mask, has_bias)
    res = run_bass_kernel_spmd(nc, in_maps, list(range(N_CORES)))
    bo = np.asarray(bo, np.float32)
    out = np.empty((B, L, EMBED), np.float32)
    for b in range(B):
        out[b] = (res.results[2 * b]["outp"].astype(np.float32)
                  + res.results[2 * b + 1]["outp"].astype(np.float32)
                  + bo[None, :])
    return out


# revision 24
# speedup vs baseline: 1.0432x; 1.0432x over previous
"""Bass/Trainium2 kernel for nn_BerpXposMultiHeadedAttention (8-core SPMD).

Sharding: data-parallel over batch (4 batches x 2 cores) x tensor-parallel over
heads (4 heads per core).  Each core computes its 4 heads of flash-style xpos
attention for its batch plus the row-sharded partial out-projection; the host
sums the two partials per batch (the "all-reduce") and adds the output bias.

v2 design notes (from NTFF trace analysis of the v1 baseline):
- v1 spent 74% of its time with the PE clock HAM-throttled to 1.2 GHz and
  another ~50us/rep on full-width DVE reciprocals.  v2 restructures for a
  dense PE stream: weights/tables load once in a preamble, x loads are 12
  full-width DMAs at rep start, and next-strip projections are emitted
  between flash head-pairs so the PE always has ready matmuls.
- Scores for a head PAIR go into one [128,1024] PSUM tile (head even in cols
  0:512 from PE rows 0:63, head odd in cols 512:1024 from rows 64:127) so
  consecutive fp32r matmuls alternate row groups and overlap in the array.
- exp (ACT) and P@V (PE) interleave per s-block: P@V for block s starts as
  soon as its exp lands, so pt tiles need only ~6 bufs and ACT/PE ping-pong
  disappears.
- Softmax normalization: denominator row -> reciprocal_approx_fast (5x
  cheaper than reciprocal, ~18 bits) -> DRAM round-trip broadcast -> one
  mul per head on GpSimd.  Drains are deferred two head-pairs to hide the
  DMA latency.
- cos|sin xpos tables are packed per strip so the post-projection combine is
  one [128,1024] DVE mul + one [128,512] add instead of three ops.
- Output is written f16 (halves the out DMA); host accumulates in f32.
"""

import sys

sys.path.insert(0, "/opt/trn_rl_repo")

import contextlib

import numpy as np

import concourse.bacc as bacc
import concourse.bass as bass
import concourse.tile as tile
from concourse import mybir
from concourse.bass_utils import run_bass_kernel_spmd

# Problem constants (hardcoded per the task contract).
B = 4
L = 2048
EMBED = 512
HEADS = 8
HD = 64
SCALE_BASE = 512
NEG = -1e9
N_CORES = 8
HPC = 4           # heads per core
TB = 512          # t-block (strip) width
NT = L // 128     # 16 t-chunks
NS = L // 128     # 16 s-chunks
NSTRIP = L // TB  # 4 strips
VW = 328          # v_aug tile width (4 heads x 65 + 68 pad)

F32 = mybir.dt.float32
F32R = mybir.dt.float32r
F16 = mybir.dt.float16
BF16 = mybir.dt.bfloat16

# Deinterleave permutation of a 64-wide head dim: evens then odds.
_PERM64 = np.concatenate([np.arange(0, HD, 2), np.arange(1, HD, 2)])


def _xpos_tables():
    """Host-side xpos cos/sin tables in the permuted [d, t] layout.

    Returns (cq, sq, ck, sk), each [128, L] float32 (two heads' worth of rows,
    identical per head).  The 1/sqrt(HD) score scale is folded into the q pair.
    """
    d = HD
    base = ((np.arange(0, d, 2, dtype=np.float32) + np.float32(0.4 * d))
            / np.float32(1.4 * d)).astype(np.float32)                    # [32]
    min_pos = -(L // 2)
    power = (np.arange(min_pos, L + min_pos, dtype=np.float32)
             / np.float32(SCALE_BASE))                                   # [L]
    scale = (base[None, :] ** power[:, None]).astype(np.float32)         # [L, 32]
    half = d // 2
    inv_freq = (1.0 / (10000.0 ** (np.arange(half, dtype=np.float32) / half))
                ).astype(np.float32)
    sinusoid = np.arange(L, dtype=np.float32)[:, None] * inv_freq[None, :]
    sin = np.sin(sinusoid).astype(np.float32)
    cos = np.cos(sinusoid).astype(np.float32)

    def pack(cs, ss, fold):
        cs = (cs * fold).astype(np.float32)
        ss = (ss * fold).astype(np.float32)
        # permuted layout: rows 0:32 <- even orig dims, rows 32:64 <- odd.
        cos_p = np.concatenate([cs.T, cs.T], axis=0)      # [64, L]
        sin_p = np.concatenate([-ss.T, ss.T], axis=0)     # [64, L]
        return (np.concatenate([cos_p, cos_p], axis=0).astype(np.float32),
                np.concatenate([sin_p, sin_p], axis=0).astype(np.float32))

    inv_scale = (1.0 / scale).astype(np.float32)
    cq, sq = pack(cos * scale, sin * scale, np.float32(HD ** -0.5))
    ck, sk = pack(cos * inv_scale, sin * inv_scale, np.float32(1.0))
    return cq, sq, ck, sk


def _build_program(causal: bool, use_mask: bool, has_bias: bool, reps: int = 1):
    nc = bacc.Bacc("TRN2", target_bir_lowering=False, debug=False,
                   num_devices=N_CORES)

    # ---- DRAM I/O -------------------------------------------------------
    xqT = nc.dram_tensor("xqT", [513, L], F16, kind="ExternalInput")
    xkT = nc.dram_tensor("xkT", [513, L], F16, kind="ExternalInput")
    xvT = nc.dram_tensor("xvT", [513, L], F16, kind="ExternalInput")
    wqcT = nc.dram_tensor("wqcT", [513, 256], BF16, kind="ExternalInput")
    wqsT = nc.dram_tensor("wqsT", [513, 256], BF16, kind="ExternalInput")
    wkcT = nc.dram_tensor("wkcT", [513, 256], BF16, kind="ExternalInput")
    wksT = nc.dram_tensor("wksT", [513, 256], BF16, kind="ExternalInput")
    wvT = nc.dram_tensor("wvT", [513, 256], BF16, kind="ExternalInput")
    woT = nc.dram_tensor("woT", [256, EMBED], BF16, kind="ExternalInput")
    # packed per-strip [cos | sin] tables: cols [tb*1024, tb*1024+512) = cos
    # strip tb, [+512, +1024) = sin strip tb.
    tqD = nc.dram_tensor("tq", [128, 2 * L], F32, kind="ExternalInput")
    tkD = nc.dram_tensor("tk", [128, 2 * L], F32, kind="ExternalInput")
    triD = nc.dram_tensor("tri", [128, 128], F32, kind="ExternalInput")
    tri01D = nc.dram_tensor("tri01", [128, 128], BF16, kind="ExternalInput")
    maskD = None
    if use_mask:
        maskD = nc.dram_tensor("maskT", [L, L], F32, kind="ExternalInput")
    outp = nc.dram_tensor("outp", [L, EMBED], F16, kind="ExternalOutput")

    xin = {"q": xqT, "k": xkT, "v": xvT}
    win = {"qc": wqcT, "qs": wqsT, "kc": wkcT, "ks": wksT, "v": wvT}

    with tile.TileContext(nc) as tc:
        with contextlib.ExitStack() as ctx:
            consts = ctx.enter_context(tc.tile_pool(name="consts", bufs=1))
            xpool = ctx.enter_context(tc.tile_pool(name="xpool", bufs=1))
            qkpool = ctx.enter_context(tc.tile_pool(name="qkpool", bufs=1))
            vpool = ctx.enter_context(tc.tile_pool(name="vpool", bufs=NS))
            tmp = ctx.enter_context(tc.tile_pool(name="tmp", bufs=3))
            ptpool = ctx.enter_context(tc.tile_pool(name="ptpool", bufs=8))
            npool = ctx.enter_context(tc.tile_pool(name="npool", bufs=6))
            opool = ctx.enter_context(tc.tile_pool(name="opool", bufs=2))
            drpool = ctx.enter_context(
                tc.tile_pool(name="drpool", bufs=6, space="DRAM"))
            mpool = None
            if use_mask:
                mpool = ctx.enter_context(tc.tile_pool(name="mpool", bufs=NS + 2))
            ps_big = ctx.enter_context(tc.tile_pool(name="ps_big", bufs=3,
                                                    space="PSUM"))
            ps_sm = ctx.enter_context(tc.tile_pool(name="ps_sm", bufs=2,
                                                   space="PSUM"))

            # ---- preamble: everything that doesn't change per rep -------
            ones_sb = None
            if has_bias:
                ones_sb = consts.tile([1, L], F16, tag="ones")
                nc.sync.dma_start(ones_sb[:], xqT[512:513, :])
            tq_sb = consts.tile([128, 2 * L], F32, tag="tq")
            tk_sb = consts.tile([128, 2 * L], F32, tag="tk")

            w_sb = {}

            def _load_w(nm):
                chunks = []
                for c in range(4):
                    t = consts.tile([128, 256], BF16, tag=f"w{nm}{c}",
                                    name=f"w{nm}{c}")
                    nc.sync.dma_start(t[:], win[nm][c * 128:(c + 1) * 128, :])
                    chunks.append(t)
                bt = None
                if has_bias:
                    bt = consts.tile([1, 256], BF16, tag=f"w{nm}b",
                                     name=f"w{nm}b")
                    nc.sync.dma_start(bt[:], win[nm][512:513, :])
                w_sb[nm] = (chunks, bt)

            # consumption order: strip-0 projections need q/k/v weights and
            # the strip-0 table slices before anything else.
            _load_w("qc")
            _load_w("qs")
            nc.sync.dma_start(tq_sb[:, 0:1024], tqD[:, 0:1024])
            _load_w("kc")
            _load_w("ks")
            nc.sync.dma_start(tk_sb[:, 0:1024], tkD[:, 0:1024])
            _load_w("v")
            tri01_sb = consts.tile([128, 128], BF16, tag="tri01")
            if causal:
                nc.sync.dma_start(tri01_sb[:], tri01D[:])
            for tb in range(1, NSTRIP):
                cs = slice(tb * 1024, (tb + 1) * 1024)
                nc.sync.dma_start(tq_sb[:, cs], tqD[:, cs])
                nc.sync.dma_start(tk_sb[:, cs], tkD[:, cs])
            wo_sb = []
            for c in range(2):
                t = consts.tile([128, EMBED], BF16, tag=f"wo{c}", name=f"wo{c}")
                nc.sync.dma_start(t[:], woT[c * 128:(c + 1) * 128, :])
                wo_sb.append(t)

            attnT = [consts.tile([128, L], BF16, tag=f"attnT{c}",
                                 name=f"attnT{c}") for c in range(2)]

            def body():
                x_sb = {}
                for nm in ("q", "k", "v"):
                    chunks = []
                    for c in range(4):
                        t = xpool.tile([128, L], F16, tag=f"x{nm}{c}",
                                       name=f"x{nm}{c}")
                        nc.sync.dma_start(t[:, 0:TB],
                                          xin[nm][c * 128:(c + 1) * 128, 0:TB])
                        chunks.append(t)
                    x_sb[nm] = chunks
                for nm in ("q", "k", "v"):
                    for c in range(4):
                        nc.sync.dma_start(
                            x_sb[nm][c][:, TB:L],
                            xin[nm][c * 128:(c + 1) * 128, TB:L])

                qTt = [[None] * NSTRIP for _ in range(2)]  # [pair][tb]
                kTt = [[None] * NSTRIP for _ in range(2)]
                vaug = [None] * NS
                pending = []

                def proj_qk(nm, tb):
                    wc, wcb = w_sb[nm + "c"]
                    ws, wsb = w_sb[nm + "s"]
                    tab = tq_sb if nm == "q" else tk_sb
                    ts = slice(tb * TB, (tb + 1) * TB)
                    dst = qTt if nm == "q" else kTt
                    for e in range(2):
                        es = slice(e * 128, (e + 1) * 128)
                        ps = ps_big.tile([128, 1024], F32, tag="s",
                                         name=f"ps_{nm}{e}_{tb}")
                        for c in range(4):
                            nc.tensor.matmul(ps[:, 0:TB], wc[c][:, es],
                                             x_sb[nm][c][:, ts],
                                             start=(c == 0),
                                             stop=(c == 3 and not has_bias))
                        if has_bias:
                            nc.tensor.matmul(ps[:, 0:TB], wcb[:, es],
                                             ones_sb[:, ts],
                                             start=False, stop=True)
                        for c in range(4):
                            nc.tensor.matmul(ps[:, TB:1024], ws[c][:, es],
                                             x_sb[nm][c][:, ts],
                                             start=(c == 0),
                                             stop=(c == 3 and not has_bias))
                        if has_bias:
                            nc.tensor.matmul(ps[:, TB:1024], wsb[:, es],
                                             ones_sb[:, ts],
                                             start=False, stop=True)
                        t12 = tmp.tile([128, 1024], F32, tag="t12",
                                       name=f"t12{nm}{e}{tb}")
                        nc.vector.tensor_mul(
                            t12[:], ps[:],
                            tab[:, tb * 1024:(tb + 1) * 1024])
                        ot = qkpool.tile([128, TB], F16, tag=f"{nm}T{e}_{tb}",
                                         name=f"{nm}T{e}_{tb}")
                        nc.vector.tensor_add(ot[:], t12[:, 0:TB],
                                             t12[:, TB:1024])
                        dst[e][tb] = ot

                def proj_v(tb, js=range(4)):
                    wv, wvb = w_sb["v"]
                    for j in js:
                        si = tb * 4 + j
                        js = slice(si * 128, (si + 1) * 128)
                        ps = ps_sm.tile([128, 256], F32, tag="pv",
                                        name=f"ps_v{si}")
                        for c in range(4):
                            nc.tensor.matmul(ps[:], x_sb["v"][c][:, js], wv[c][:],
                                             start=(c == 0),
                                             stop=(c == 3 and not has_bias))
                        if has_bias:
                            nc.tensor.matmul(
                                ps[:], ones_sb[:, js],
                                wvb[:], start=False, stop=True)
                        va = vpool.tile([128, VW], BF16, tag="vaug",
                                        name=f"vaug{si}")
                        va3 = va[:, 0:HPC * 65].rearrange("p (h c) -> p h c", c=65)
                        nc.vector.tensor_copy(
                            va3[:, :, 0:64],
                            ps[:].rearrange("p (h d) -> p h d", d=64))
                        nc.gpsimd.memset(va3[:, :, 64:65], 1.0)
                        nc.gpsimd.memset(va[:, HPC * 65:VW], 0.0)
                        vaug[si] = va

                def launch_norm(po, ht, hr, T):
                    poc = npool.tile([65, TB], F32, tag="poc",
                                     name=f"poc{T}h{ht}{hr}", bufs=6)
                    nc.vector.tensor_copy(poc[:], po[0:65, :])
                    dbn = drpool.tile([1, TB], F32, tag="dbn",
                                      name=f"dbn{T}h{ht}{hr}")
                    nc.sync.dma_start(dbn[:], poc[64:65, :])
                    sums = npool.tile([64, TB], F32, tag="sums",
                                      name=f"sums{T}h{ht}{hr}", bufs=6)
                    bcast = bass.AP(tensor=dbn[:].tensor, offset=dbn[:].offset,
                                    ap=[[0, 64], [1, TB]])
                    nc.sync.dma_start(sums[:], bcast)
                    pending.append((poc, sums, ht, hr, T))

                def drain_norms(keep=0):
                    while len(pending) > keep:
                        poc, sums, ht, hr, T = pending.pop(0)
                        tcols = slice(T * TB, (T + 1) * TB)
                        rec = npool.tile([64, TB], F32, tag="recr",
                                         name=f"rec{T}x{ht}{hr}", bufs=2)
                        nc.vector.reciprocal_approx_fast(rec[:], sums[:])
                        if hr == 0:
                            nc.gpsimd.tensor_mul(attnT[ht][0:64, tcols],
                                                 poc[0:64, :], rec[:])
                        else:
                            stag = npool.tile([64, TB], BF16, tag="stag",
                                              name=f"stag{T}x{ht}", bufs=2)
                            nc.gpsimd.tensor_mul(stag[:], poc[0:64, :], rec[:])
                            nc.sync.dma_start(attnT[ht][64:128, tcols], stag[:])

                def flash_strip(T):
                    nsig = 4 * T + 4 if causal else NS
                    mtiles = None
                    if use_mask:
                        mtiles = []
                        for si in range(nsig):
                            mt = mpool.tile([128, TB], F32, tag="mask",
                                            name=f"m{T}_{si}")
                            nc.sync.dma_start(
                                mt[:], maskD[si * 128:(si + 1) * 128,
                                             T * TB:(T + 1) * TB])
                            mtiles.append(mt)
                    for hp in range(2):
                        drain_norms(keep=4 if T < NSTRIP - 1 else 2)
                        # keep the PE fed with next-strip projection work
                        # while ACT chews on this pair's exps.
                        if T < NSTRIP - 1:
                            if hp == 0:
                                proj_qk("q", T + 1)
                                proj_v(T + 1, range(0, 2))
                            else:
                                proj_qk("k", T + 1)
                                proj_v(T + 1, range(2, 4))
                        elif hp == 1:
                            # strip 2's second pair must be normalized before
                            # its out batch reads those attnT columns.
                            drain_norms(keep=1)
                            out_proj_batch(2)
                            out_proj_batch(1)
                        hA, hB = 2 * hp, 2 * hp + 1
                        poA = ps_sm.tile([128, TB], F32, tag="pv",
                                         name=f"poA{T}p{hp}")
                        poB = ps_sm.tile([128, TB], F32, tag="pv",
                                         name=f"poB{T}p{hp}")
                        for sig in range(nsig):
                            ps2 = ps_big.tile([128, 1024], F32, tag="s",
                                              name=f"S{T}p{hp}s{sig}")
                            pt = ptpool.tile([128, 1024], BF16, tag="pt",
                                             name=f"P{T}p{hp}s{sig}")
                            j = sig - 4 * T
                            coff = 0
                            if causal and j >= 0:
                                ncols = max(TB - j * 128, 256)
                                coff = TB - ncols
                            kT_s = kTt[hp][sig // 4]
                            qT_s = qTt[hp][T]
                            ss = slice((sig % 4) * 128, (sig % 4 + 1) * 128)
                            nc.tensor.matmul(ps2[:, coff:TB],
                                             kT_s[0:64, ss],
                                             qT_s[0:64, coff:TB],
                                             start=True, stop=True)
                            nc.tensor.matmul(ps2[:, TB + coff:1024],
                                             kT_s[64:128, ss],
                                             qT_s[64:128, coff:TB],
                                             start=True, stop=True)
                            if use_mask:
                                for u in range(2):
                                    sl = slice(u * TB, (u + 1) * TB)
                                    nc.vector.tensor_add(ps2[:, sl], ps2[:, sl],
                                                         mtiles[sig][:])
                            # exp only the matmul-written region: reading the
                            # unwritten [0:coff) would race the pool slot's
                            # next occupant (same-bank PE-W vs ACT-R).
                            pt3 = pt[:].rearrange("p (u t) -> p u t", u=2)
                            ps3 = ps2[:].rearrange("p (u t) -> p u t", u=2)
                            nc.scalar.activation(pt3[:, :, coff:TB],
                                                 ps3[:, :, coff:TB],
                                                 mybir.ActivationFunctionType.Exp)
                            for u in range(2):
                                if causal and j >= 0:
                                    sl = slice(u * TB + j * 128,
                                               u * TB + (j + 1) * 128)
                                    nc.gpsimd.tensor_mul(pt[:, sl], pt[:, sl],
                                                         tri01_sb[:])
                                if causal and 1 <= j <= 3:
                                    nc.gpsimd.memset(
                                        pt[:, u * TB:u * TB + j * 128], 0.0)
                            nc.tensor.matmul(
                                poA[:], vaug[sig][:, hA * 65:hA * 65 + 128],
                                pt[:, 0:TB],
                                start=(sig == 0), stop=(sig == nsig - 1))
                            nc.tensor.matmul(
                                poB[:], vaug[sig][:, hB * 65:hB * 65 + 128],
                                pt[:, TB:1024],
                                start=(sig == 0), stop=(sig == nsig - 1))
                        launch_norm(poA, hp, 0, T)
                        launch_norm(poB, hp, 64, T)
                    # attnT cols for strip T-2 are fully normalized once this
                    # strip's pairs have drained; emit its out batch now.
                    if T == 2:
                        out_proj_batch(0)

                def out_proj_batch(ob):
                    osb = opool.tile([128, 4 * EMBED], F16, tag="osb",
                                     name=f"osb{ob}")
                    for i in range(4):
                        tau = ob * 4 + i
                        ps = ps_sm.tile([128, EMBED], F32, tag="pv",
                                        name=f"ps_o{tau}")
                        for c in range(2):
                            nc.tensor.matmul(
                                ps[:], attnT[c][:, tau * 128:(tau + 1) * 128],
                                wo_sb[c][:], start=(c == 0), stop=(c == 1))
                        nc.vector.tensor_copy(
                            osb[:, i * EMBED:(i + 1) * EMBED], ps[:])
                    dst = outp[ob * 512:(ob + 1) * 512, :].rearrange(
                        "(i p) e -> p i e", i=4)
                    nc.sync.dma_start(
                        dst, osb[:].rearrange("p (i e) -> p i e", i=4))

                proj_qk("q", 0)
                proj_qk("k", 0)
                proj_v(0)
                for T in range(NSTRIP):
                    flash_strip(T)
                drain_norms()
                out_proj_batch(3)

            if reps > 1:
                inner = 4 if reps % 4 == 0 else (2 if reps % 2 == 0 else 1)
                with tc.For_i(0, reps // inner, 1,
                              hint_engines=(mybir.EngineType.PE,
                                            mybir.EngineType.Activation,
                                            mybir.EngineType.DVE,
                                            mybir.EngineType.SP,
                                            mybir.EngineType.Pool)):
                    for _ in range(inner):
                        body()
            else:
                body()

    nc.compile()
    return nc


_PROGRAM_CACHE = {}


def get_program(causal: bool, use_mask: bool, has_bias: bool, reps: int = 1):
    key = (causal, use_mask, has_bias, reps)
    if key not in _PROGRAM_CACHE:
        _PROGRAM_CACHE[key] = _build_program(causal, use_mask, has_bias, reps)
    return _PROGRAM_CACHE[key]


def _prep_in_maps(query, key, value, key_padding_mask, attn_mask,
                  Wq, bq, Wk, bk, Wv, bv, Wo, bo, use_mask, has_bias):
    """Build the 8 per-core input dicts."""
    import ml_dtypes
    cq, sq, ck, sk = _xpos_tables()
    # pack per strip: [cos strip | sin strip] -> [128, 2L]
    tq = np.empty((128, 2 * L), np.float32)
    tk = np.empty((128, 2 * L), np.float32)
    for tb in range(NSTRIP):
        ts = slice(tb * TB, (tb + 1) * TB)
        tq[:, tb * 1024:tb * 1024 + TB] = cq[:, ts]
        tq[:, tb * 1024 + TB:(tb + 1) * 1024] = sq[:, ts]
        tk[:, tb * 1024:tb * 1024 + TB] = ck[:, ts]
        tk[:, tb * 1024 + TB:(tb + 1) * 1024] = sk[:, ts]
    tri = np.where(np.arange(128)[None, :] >= np.arange(128)[:, None],
                   np.float32(0.0), np.float32(NEG)).astype(np.float32)
    import ml_dtypes as _mld
    tri01 = np.where(np.arange(128)[None, :] >= np.arange(128)[:, None],
                     np.float32(1.0), np.float32(0.0)).astype(_mld.bfloat16)

    def aug_x(x):
        a = np.empty((513, L), np.float16)
        a[0:512] = np.asarray(x, np.float32).T.astype(np.float16)
        a[512] = np.float16(1.0)
        return a

    xqTs = [aug_x(query[b]) for b in range(B)]
    xkTs = [aug_x(key[b]) for b in range(B)]
    xvTs = [aug_x(value[b]) for b in range(B)]

    masks = None
    if use_mask:
        am = np.asarray(attn_mask, np.float32)
        kp = np.asarray(key_padding_mask)
        masks = []
        for b in range(B):
            m = am.copy()
            if kp[b].any():
                m = m + np.where(kp[b], np.float32(-1e30),
                                 np.float32(0.0))[None, :]
            masks.append(np.ascontiguousarray(m.T.astype(np.float32)))

    Wq = np.asarray(Wq, np.float32); bq = np.asarray(bq, np.float32)
    Wk = np.asarray(Wk, np.float32); bk = np.asarray(bk, np.float32)
    Wv = np.asarray(Wv, np.float32); bv = np.asarray(bv, np.float32)
    Wo = np.asarray(Wo, np.float32)

    in_maps = []
    for core in range(N_CORES):
        b, hg = core // 2, core % 2
        hs = hg * HPC
        idx_p = np.concatenate(
            [hs * HD + hl * HD + _PERM64 for hl in range(HPC)])
        # sin-projection rows: within each head's 64-block, row r <- r XOR 32
        xor = (np.arange(256).reshape(HPC, HD)[:, (np.arange(HD) ^ 32)]
               ).reshape(-1)
        idx_s = idx_p[xor]
        idx_v = hs * HD + np.arange(HPC * HD)

        def aug_w(W, bias, idx):
            a = np.empty((513, 256), np.float32)
            a[0:512] = np.ascontiguousarray(W[idx, :].T)
            a[512] = bias[idx]
            return a.astype(ml_dtypes.bfloat16)

        m = {
            "xqT": xqTs[b], "xkT": xkTs[b], "xvT": xvTs[b],
            "wqcT": aug_w(Wq, bq, idx_p),
            "wqsT": aug_w(Wq, bq, idx_s),
            "wkcT": aug_w(Wk, bk, idx_p),
            "wksT": aug_w(Wk, bk, idx_s),
            "wvT": aug_w(Wv, bv, idx_v),
            "woT": np.ascontiguousarray(Wo[:, idx_v].T).astype(ml_dtypes.bfloat16),
            "tq": tq, "tk": tk,
            "tri": tri, "tri01": tri01,
        }
        if use_mask:
            m["maskT"] = masks[b]
        in_maps.append(m)
    return in_maps


def classify_mask(attn_mask, key_padding_mask):
    am = np.asarray(attn_mask, np.float32)
    kp = np.asarray(key_padding_mask)
    if not kp.any():
        causal = np.where(
            np.tril(np.ones((L, L), bool)), np.float32(0.0),
            np.float32(NEG)).astype(np.float32)
        if np.array_equal(am, causal):
            return True, False
        if not am.any():
            return False, False
    return False, True


def kernel(query, key, value, key_padding_mask, attn_mask,
           Wq, bq, Wk, bk, Wv, bv, Wo, bo):
    causal, use_mask = classify_mask(attn_mask, key_padding_mask)
    has_bias = bool(np.asarray(bq).any() or np.asarray(bk).any()
                    or np.asarray(bv).any())
    nc = get_program(causal, use_mask, has_bias, reps=1)
    in_maps = _prep_in_maps(query, key, value, key_padding_mask, attn_mask,
                            Wq, bq, Wk, bk, Wv, bv, Wo, bo, use_mask, has_bias)
    res = run_bass_kernel_spmd(nc, in_maps, list(range(N_CORES)))
    bo = np.asarray(bo, np.float32)
    out = np.empty((B, L, EMBED), np.float32)
    for b in range(B):
        out[b] = (res.results[2 * b]["outp"].astype(np.float32)
                  + res.results[2 * b + 1]["outp"].astype(np.float32)
                  + bo[None, :])
    return out
